# revision 2
# baseline (speedup 1.0000x reference)
"""BandSplit kernel v4 for Trainium2 (8 NeuronCores, SPMD data-parallel).

Math: the (deterministic) melbank partitions the 1025 STFT bins into 257
contiguous segments (widths 1/4/8/8/1), all mel weights are 1.0, so

    out[b,c,t,k,o] = sum_{f in seg(k)} sum_i x[b,c,t,f,i]*pre_w[i,f,o] + pre_b[k,o]

Sharding: data-parallel over the 8 (b,c) pairs, one per core.

v4 design (vs v3 baseline):
- x is loaded DENSE: xmm = x.T folded to (128, 17*256) bf16, 1.11 MB
  (v3 packed mm-aligned copies + ones rows: 2.42 MB). Matmuls read
  full 32-row windows (K=32) of the dense layout; the block-diagonal
  band structure lives entirely in the w operand.
- bias is added by PSUM-priming matmuls (K=1: ones-row lhsT x bias-row
  rhs, start=True) before the data matmuls accumulate (start=False).
  Bias/ones rows live in a tiny (4, ~9K) strip loaded per off-row.
- all input loads issue from the gpsimd (SWDGE) queue at the very top
  so the Scalar/Vector engines do nothing but PSUM drains and the sync
  (HWDGE) queue does nothing but output chunks.
- drains are (128, 2048) (4 PSUM banks = one quad), alternating
  Vector/Scalar; one 512 KB output DMA per quad.
Total HBM/core ~20.1 MB (3.3 in + 16.8 out) vs 21.3 for v3.
"""

import numpy as np
import ml_dtypes

import concourse.bacc as bacc
import concourse.mybir as mybir
from concourse.tile import TileContext
from concourse.bass_utils import run_bass_kernel_spmd

BF16 = np.dtype(ml_dtypes.bfloat16)

B, C, T, NF, IN_CH = 4, 2, 256, 1025, 2
N_BANDS, OUT_CH = 257, 128
N_CORES = 8
TOK = 256
HALVES = 2

NG = 17                    # dense x groups (2176 rows = 17*128)
XCOLS = NG * TOK           # 4352
WC_A, WC_B, WC_C = 4096, 1024, 3072
WB_B, WB_C, WB_R = WC_A, WC_A + WC_B, WC_A + WC_B + WC_C   # col bases
WCOLS = WC_A + WC_B + WC_C + 128                           # 8320

# ---------------------------------------------------------------- plan
# A bank: one mm  (window g1 in {0,1}, g0, m in 0..3): K=32 N=512, 4 bands
# B bank: one mm  (g1 in {2,3}, g0):                   K=32 N=512, 4 bands
# C bank: two mms (windows (g1,g0),(g1+1,g0)):         K=32 N=256 x2, 4 bands
# runt bank: one mm K=2 N=128 (band 256)
# bank = dict(off, mms=[(row0, K, wcol, N)...], bands=[...], N=512|128)


def _build_banks():
    by_off = [[] for _ in range(4)]
    for g1 in range(2):
        for m in range(4):
            for g0 in range(4):
                k0 = 64 * g1 + 16 * g0 + 4 * m
                by_off[g0].append(dict(
                    off=32 * g0,
                    mms=[(128 * g1 + 32 * g0, 32, 2048 * g1 + 512 * m, 512)],
                    bands=[k0, k0 + 1, k0 + 2, k0 + 3], N=512))
    for g1 in (2, 3):
        for g0 in range(4):
            k0 = 128 + 16 * (g1 - 2) + 4 * g0
            by_off[g0].append(dict(
                off=32 * g0,
                mms=[(128 * g1 + 32 * g0, 32, WB_B + 512 * (g1 - 2), 512)],
                bands=[k0, k0 + 1, k0 + 2, k0 + 3], N=512))
    for t in range(6):
        for g0 in range(4):
            g1a, g1b = 4 + 2 * t, 5 + 2 * t
            ka = 160 + 8 * (g1a - 4) + 2 * g0
            kb = 160 + 8 * (g1b - 4) + 2 * g0
            by_off[g0].append(dict(
                off=32 * g0,
                mms=[(128 * g1a + 32 * g0, 32, WB_C + 256 * (g1a - 4), 256),
                     (128 * g1b + 32 * g0, 32, WB_C + 256 * (g1b - 4), 256)],
                bands=[ka, ka + 1, kb, kb + 1], N=512))
    runt = dict(off=0, mms=[(128 * 16, 2, WB_R, 128)], bands=[256], N=128)
    # quads: rotate offs for PE row-group concurrency
    quads = []
    for i in range(16):
        quads.append([by_off[0][i], by_off[1][i], by_off[2][i], by_off[3][i]])
    by_off[0].append(runt)
    # bias strip cols per off
    bias_cols = [0, 0, 0, 0]
    for o in range(4):
        for bk in by_off[o]:
            bk["bias_col"] = bias_cols[o]
            bias_cols[o] += bk["N"]
    ones_col = max(bias_cols)
    bscols = ones_col + 128
    return quads, runt, bscols, ones_col


QUADS, RUNT, BSCOLS, ONES_COL = _build_banks()

# band order in output emission order (for host assembly)
_BAND_ORDER = []
for _h in range(HALVES):
    for _q in QUADS:
        for _bk in _q:
            _BAND_ORDER.extend(_bk["bands"])
    _BAND_ORDER.extend(RUNT["bands"])
_BAND_PERM = np.array(_BAND_ORDER[:N_BANDS * 1])  # same order per half
OELEMS = TOK * N_BANDS * OUT_CH

# ---------------------------------------------------------------- host prep


def _build_wmm(pre_w):
    """(128, WCOLS) bf16 block-diagonal weights (no bias rows)."""
    wmm = np.zeros((128, WCOLS), dtype=np.float32)
    for g1 in range(2):          # class A
        for g0 in range(4):
            for j in range(16):
                f = 64 * g1 + 16 * g0 + j
                for r in range(2):
                    wmm[32 * g0 + 2 * j + r,
                        2048 * g1 + 128 * j: 2048 * g1 + 128 * (j + 1)] = \
                        pre_w[r, f, :]
    for g1 in (2, 3):            # class B
        for g0 in range(4):
            for b in range(4):
                for rr in range(8):
                    f = 64 * g1 + 16 * g0 + 4 * b + rr // 2
                    wmm[32 * g0 + 8 * b + rr,
                        WB_B + 512 * (g1 - 2) + 128 * b:
                        WB_B + 512 * (g1 - 2) + 128 * (b + 1)] = \
                        pre_w[rr % 2, f, :]
    for g1 in range(4, 16):      # class C
        for g0 in range(4):
            for b in range(2):
                for rr in range(16):
                    f = 64 * g1 + 16 * g0 + 8 * b + rr // 2
                    wmm[32 * g0 + 16 * b + rr,
                        WB_C + 256 * (g1 - 4) + 128 * b:
                        WB_C + 256 * (g1 - 4) + 128 * (b + 1)] = \
                        pre_w[rr % 2, f, :]
    for r in range(2):           # runt
        wmm[r, WB_R:WB_R + 128] = pre_w[r, 1024, :]
    return wmm.astype(BF16)


def _build_bias_strip(pre_b):
    """(4, BSCOLS) bf16: per-off bank bias rows + trailing ones block."""
    bs = np.zeros((4, BSCOLS), dtype=np.float32)
    for o in range(4):
        banks = [q[o] for q in QUADS]
        if o == 0:
            banks = banks + [RUNT]
        for bk in banks:
            c = bk["bias_col"]
            for j, k in enumerate(bk["bands"]):
                bs[o, c + 128 * j: c + 128 * (j + 1)] = pre_b[k, :]
    bs[:, ONES_COL:ONES_COL + 128] = 1.0
    return bs.astype(BF16)


def _build_combo(xmm, wmm, bs):
    """(128, 1408) bf16: quad-0 working set in one transfer."""
    cb = np.zeros((128, 1408), dtype=BF16)
    cb[:, 0:256] = xmm[:, 0:256]
    cb[:, 256:768] = wmm[:, 0:512]
    for o in range(4):
        cb[32 * o, 768:1280] = bs[o, 0:512]
        cb[32 * o, 1280:1408] = np.array(1.0, dtype=BF16)
    return cb


def _build_xmm(x_core):
    """x_core (TOK, NF, IN_CH) -> (128, XCOLS) bf16 dense x.T layout."""
    xt = x_core.reshape(TOK, NF * IN_CH).T  # (2050, TOK)
    xp = np.zeros((NG * 128, TOK), dtype=np.float32)
    xp[:2050] = xt
    return np.ascontiguousarray(
        xp.reshape(NG, 128, TOK).transpose(1, 0, 2)).reshape(
            128, XCOLS).astype(BF16)


_INV_PERM = np.argsort(_BAND_PERM)


def _assemble(out_flat):
    """flat device output (bf16) -> (TOK, N_BANDS, OUT_CH) fp32.

    Device layout per half: 16 chunks of (128, 2048) then one (128, 128)
    runt chunk, chunk-major. Stitch cols back together, then un-permute
    the band order."""
    half = out_flat.reshape(2, 128 * 32896)
    chunk_cols = [2048] + [4096] * 7 + [2048, 128]
    oc = np.empty((TOK, N_BANDS, OUT_CH), dtype=np.float32)
    for h in range(2):
        flat = half[h]
        blk = np.empty((128, 32896), dtype=np.float32)
        c = 0
        o = 0
        for nc_ in chunk_cols:
            blk[:, c:c + nc_] = flat[o:o + 128 * nc_].reshape(128, nc_)
            c += nc_
            o += 128 * nc_
        blk3 = blk.reshape(128, N_BANDS, OUT_CH)
        oc[128 * h:128 * (h + 1)] = blk3[:, _INV_PERM, :]
    return oc


# ---------------------------------------------------------------- device

_PROGRAM = None


def _build_program():
    global _PROGRAM
    if _PROGRAM is not None:
        return _PROGRAM

    nc = bacc.Bacc("TRN2", target_bir_lowering=False)
    f32 = mybir.dt.float32
    bf16 = mybir.dt.bfloat16
    xin = nc.dram_tensor("xmm", [128, XCOLS], bf16, kind="ExternalInput")
    win = nc.dram_tensor("wmm", [128, WCOLS], bf16, kind="ExternalInput")
    bsin = nc.dram_tensor("bs", [4, BSCOLS], bf16, kind="ExternalInput")
    cbin = nc.dram_tensor("combo", [128, 1408], bf16, kind="ExternalInput")
    out = nc.dram_tensor("out", [TOK * N_BANDS * OUT_CH], bf16,
                         kind="ExternalOutput")

    with TileContext(nc) as tc:
        with (
            tc.tile_pool(name="xw", bufs=1) as xw_pool,
            tc.tile_pool(name="stage", bufs=4) as stage_pool,
            tc.tile_pool(name="psum", bufs=4, space="PSUM") as psum_pool,
        ):
            x_sb = xw_pool.tile([128, XCOLS], bf16, tag="x")
            w_sb = xw_pool.tile([128, WCOLS], bf16, tag="w")
            bs_sb = xw_pool.tile([128, BSCOLS], bf16, tag="bs")
            cb_sb = xw_pool.tile([128, 1408], bf16, tag="cb")

            # Loads in first-needed-first order. The whole first chunk's
            # working set (x group 0, the A/g1=0/m=0 w block, quad-0 bias
            # rows + ones) arrives in ONE sync DMA (the "combo"), so the
            # first matmul waits on exactly one transfer. Everything else
            # streams on gpsimd (SWDGE) in quad order; the drain engines
            # and the output queue never issue loads.
            nc.sync.dma_start(out=cb_sb[:], in_=cbin.ap()[:])
            nc.sync.dma_start(out=x_sb[:, 0:256], in_=xin.ap()[:, 0:256])
            bs_dst = bs_sb[:].rearrange("(a b) c -> a b c", a=4)[:, 0:1, :]
            nc.gpsimd.dma_start(out=bs_dst,
                                in_=bsin.ap()[:].rearrange(
                                    "a (b c) -> a b c", b=1))
            nc.gpsimd.dma_start(out=w_sb[:, 512:2048],
                                in_=win.ap()[:, 512:2048])
            nc.gpsimd.dma_start(out=x_sb[:, 256:1024],
                                in_=xin.ap()[:, 256:1024])
            nc.gpsimd.dma_start(out=w_sb[:, 2048:5120],
                                in_=win.ap()[:, 2048:5120])
            nc.gpsimd.dma_start(out=x_sb[:, 1024:XCOLS],
                                in_=xin.ap()[:, 1024:XCOLS])
            nc.gpsimd.dma_start(out=w_sb[:, 5120:WCOLS],
                                in_=win.ap()[:, 5120:WCOLS])

            def emit_bank(ps, pc, bk, h, combo=False):
                off = bk["off"]
                tp = (off, 0)
                if combo:
                    ones = cb_sb[off:off + 1, 1280:1408]
                    bias = cb_sb[off:off + 1, 768:768 + bk["N"]]
                else:
                    ones = bs_sb[off:off + 1, ONES_COL:ONES_COL + 128]
                    bias = bs_sb[off:off + 1,
                                 bk["bias_col"]:bk["bias_col"] + bk["N"]]
                # prime bias (start resets PSUM)
                nc.tensor.matmul(
                    ps[:, pc:pc + bk["N"]], ones, bias,
                    start=True, stop=False, tile_position=tp,
                    skip_group_check=True,
                )
                c = pc
                for (r0, K, wcol, N) in bk["mms"]:
                    g = r0 // 128
                    row = r0 % 128
                    last = (c + N == pc + bk["N"])
                    if combo:
                        lhsT = cb_sb[row:row + K, 128 * h:128 * (h + 1)]
                        rhs = cb_sb[row:row + K, 256 + (wcol - 0):
                                    256 + (wcol - 0) + N]
                    else:
                        lhsT = x_sb[row:row + K, 256 * g + 128 * h:
                                    256 * g + 128 * (h + 1)]
                        rhs = w_sb[row:row + K, wcol:wcol + N]
                    nc.tensor.matmul(
                        ps[:, c:c + N], lhsT, rhs,
                        start=False, stop=last, tile_position=tp,
                        skip_group_check=True,
                    )
                    c += N

            # chunk grouping: first chunk = quad 0 alone (fast stream
            # start), then pairs of quads (1 MB chunks, half the DMA +
            # semaphore count of per-quad chunks).
            groups = [[0]] + [[1 + 2 * i, 2 + 2 * i] for i in range(7)] \
                + [[15]]
            o = 0
            drain_load = [0, 0]  # [dve_tiles, act_tiles]
            for h in range(HALVES):
                for grp in groups:
                    ncols = 2048 * len(grp)
                    sb = stage_pool.tile([128, ncols], bf16, tag="st",
                                         name="sq")
                    for gi, qi in enumerate(grp):
                        quad = QUADS[qi]
                        pts = [psum_pool.tile([128, 1024], f32, tag="ps",
                                              name="pq") for _ in range(2)]
                        for bi, bk in enumerate(quad):
                            emit_bank(pts[bi // 2], 512 * (bi % 2), bk, h,
                                      combo=(qi == 0))
                        for ti in range(2):
                            # greedy balance by measured per-tile cost
                            # (DVE 1.21us, ACT 1.05us)
                            if drain_load[0] * 1.21 <= drain_load[1] * 1.05:
                                drain_load[0] += 1
                                use_dve = True
                            else:
                                drain_load[1] += 1
                                use_dve = False
                            dst = sb[:, 2048 * gi + 1024 * ti:
                                     2048 * gi + 1024 * (ti + 1)]
                            if use_dve:
                                nc.vector.tensor_copy(dst, pts[ti][:])
                            else:
                                nc.scalar.copy(dst, pts[ti][:])
                    nc.sync.dma_start(
                        out=out.ap()[o:o + 128 * ncols]
                            .rearrange("(p n) -> p n", n=ncols),
                        in_=sb[:],
                    )
                    o += 128 * ncols
                # runt bank
                ps = psum_pool.tile([128, 128], f32, tag="ps", name="pr")
                emit_bank(ps, 0, RUNT, h)
                sb = stage_pool.tile([128, 128], bf16, tag="st", name="sr")
                nc.scalar.copy(sb[:], ps[:])
                nc.sync.dma_start(
                    out=out.ap()[o:o + 128 * 128]
                        .rearrange("(p n) -> p n", n=128),
                    in_=sb[:],
                )
                o += 128 * 128

    nc.compile()
    _PROGRAM = nc
    return nc


# ---------------------------------------------------------------- entry

LAST_RESULTS = None


def kernel(x, pre_w, pre_b, _trace=False):
    global LAST_RESULTS
    x = np.asarray(x, dtype=np.float32)
    pre_w = np.asarray(pre_w, dtype=np.float32)
    pre_b = np.asarray(pre_b, dtype=np.float32)
    assert x.shape == (B, C, T, NF, IN_CH), x.shape

    nc = _build_program()
    wmm = _build_wmm(pre_w)
    bs = _build_bias_strip(pre_b)
    in_maps = []
    for core in range(N_CORES):
        b_, c_ = divmod(core, C)
        xmm = _build_xmm(x[b_, c_])
        in_maps.append({"xmm": xmm, "wmm": wmm, "bs": bs,
                        "combo": _build_combo(xmm, wmm, bs)})

    res = run_bass_kernel_spmd(
        nc, in_maps, core_ids=list(range(N_CORES)), trace=_trace,
    )
    LAST_RESULTS = res

    out = np.empty((B, C, T, N_BANDS, OUT_CH), dtype=np.float32)
    for core in range(N_CORES):
        b_, c_ = divmod(core, C)
        out[b_, c_] = _assemble(res.results[core]["out"])
    return out


# revision 7
# speedup vs baseline: 1.0098x; 1.0098x over previous
"""BandSplit kernel v4.5 for Trainium2 (8 NeuronCores, SPMD data-parallel).

Math: the (deterministic) melbank partitions the 1025 STFT bins into 257
contiguous segments (widths 1/4/8/8/1), all mel weights are 1.0, so

    out[b,c,t,k,o] = sum_{f in seg(k)} sum_i x[b,c,t,f,i]*pre_w[i,f,o] + pre_b[k,o]

Sharding: data-parallel over the 8 (b,c) pairs, one per core.
The stream is HBM-bound (358 GB/s/NC): ~20.1 MB/core (3.3 in + 16.8
out) vs 21.3 for the previous version; measured ~11% faster than it
under matched conditions (interleaved A/B on the same device state).

Design:
- x is loaded DENSE: xmm = x.T folded to (128, 17*256) bf16, 1.11 MB
  (previously packed per-mm copies + ones rows: 2.42 MB). Matmuls read
  full 32-row windows (K=32) of the dense layout; the block-diagonal
  band structure lives entirely in the w operand (32-aligned bases are
  a hard BIR-verifier rule, so w keeps its zero padding).
- bias is added by PSUM-priming matmuls (K=1 ones-row lhsT x bias-row
  rhs, start=True) before the data matmuls accumulate (start=False).
  Bias/ones rows live in a (4, ~9K) strip at partitions {0,32,64,96}.
- the ENTIRE first-chunk working set (x group 0, the A/g1=0 w block,
  quad-0 bias+ones rows) arrives in ONE "combo" DMA on sync, so the
  first matmul waits on exactly one transfer (~2 us DMA fixed latency
  dominates the ramp; serializing 3 loads cost ~2.5 us extra before).
- bulk loads stream on gpsimd (SWDGE) in quad order; Scalar/Vector do
  nothing but PSUM drains (greedy-balanced: ACT ~1.05 us vs DVE
  ~1.21 us per (128,1024) fp32->bf16 drain); sync does only output.
- output chunks: first = one (128,1024) PSUM tile (first DMA issues
  after a single drain), then a 3-tile chunk, then 1 MB 4-tile chunks
  (halves the DMA + semaphore count -> shorter epilogue).
Rel err ~5.5e-3 (bf16 I/O rounding), inside the 2e-2 gate.
"""

import numpy as np
import ml_dtypes

import concourse.bacc as bacc
import concourse.mybir as mybir
from concourse.tile import TileContext
from concourse.bass_utils import run_bass_kernel_spmd

BF16 = np.dtype(ml_dtypes.bfloat16)

B, C, T, NF, IN_CH = 4, 2, 256, 1025, 2
N_BANDS, OUT_CH = 257, 128
N_CORES = 8
TOK = 256
HALVES = 2

NG = 17                    # dense x groups (2176 rows = 17*128)
XCOLS = NG * TOK           # 4352
WC_A, WC_B, WC_C = 4096, 1024, 3072
WB_B, WB_C, WB_R = WC_A, WC_A + WC_B, WC_A + WC_B + WC_C   # col bases
WCOLS = WC_A + WC_B + WC_C + 128                           # 8320

# ---------------------------------------------------------------- plan
# A bank: one mm  (window g1 in {0,1}, g0, m in 0..3): K=32 N=512, 4 bands
# B bank: one mm  (g1 in {2,3}, g0):                   K=32 N=512, 4 bands
# C bank: two mms (windows (g1,g0),(g1+1,g0)):         K=32 N=256 x2, 4 bands
# runt bank: one mm K=2 N=128 (band 256)
# bank = dict(off, mms=[(row0, K, wcol, N)...], bands=[...], N=512|128)


def _build_banks():
    by_off = [[] for _ in range(4)]
    for g1 in range(2):
        for m in range(4):
            for g0 in range(4):
                k0 = 64 * g1 + 16 * g0 + 4 * m
                by_off[g0].append(dict(
                    off=32 * g0,
                    mms=[(128 * g1 + 32 * g0, 32, 2048 * g1 + 512 * m, 512)],
                    bands=[k0, k0 + 1, k0 + 2, k0 + 3], N=512))
    for g1 in (2, 3):
        for g0 in range(4):
            k0 = 128 + 16 * (g1 - 2) + 4 * g0
            by_off[g0].append(dict(
                off=32 * g0,
                mms=[(128 * g1 + 32 * g0, 32, WB_B + 512 * (g1 - 2), 512)],
                bands=[k0, k0 + 1, k0 + 2, k0 + 3], N=512))
    for t in range(6):
        for g0 in range(4):
            g1a, g1b = 4 + 2 * t, 5 + 2 * t
            ka = 160 + 8 * (g1a - 4) + 2 * g0
            kb = 160 + 8 * (g1b - 4) + 2 * g0
            by_off[g0].append(dict(
                off=32 * g0,
                mms=[(128 * g1a + 32 * g0, 32, WB_C + 256 * (g1a - 4), 256),
                     (128 * g1b + 32 * g0, 32, WB_C + 256 * (g1b - 4), 256)],
                bands=[ka, ka + 1, kb, kb + 1], N=512))
    runt = dict(off=0, mms=[(128 * 16, 2, WB_R, 128)], bands=[256], N=128)
    # quads: rotate offs for PE row-group concurrency
    quads = []
    for i in range(16):
        quads.append([by_off[0][i], by_off[1][i], by_off[2][i], by_off[3][i]])
    by_off[0].append(runt)
    # bias strip cols per off
    bias_cols = [0, 0, 0, 0]
    for o in range(4):
        for bk in by_off[o]:
            bk["bias_col"] = bias_cols[o]
            bias_cols[o] += bk["N"]
    ones_col = max(bias_cols)
    bscols = ones_col + 128
    return quads, runt, bscols, ones_col


QUADS, RUNT, BSCOLS, ONES_COL = _build_banks()

# band order in output emission order (for host assembly)
_BAND_ORDER = []
for _h in range(HALVES):
    for _q in QUADS:
        for _bk in _q:
            _BAND_ORDER.extend(_bk["bands"])
    _BAND_ORDER.extend(RUNT["bands"])
_BAND_PERM = np.array(_BAND_ORDER[:N_BANDS * 1])  # same order per half
OELEMS = TOK * N_BANDS * OUT_CH

# ---------------------------------------------------------------- host prep


def _build_wmm(pre_w):
    """(128, WCOLS) bf16 block-diagonal weights (no bias rows)."""
    wmm = np.zeros((128, WCOLS), dtype=np.float32)
    for g1 in range(2):          # class A
        for g0 in range(4):
            for j in range(16):
                f = 64 * g1 + 16 * g0 + j
                for r in range(2):
                    wmm[32 * g0 + 2 * j + r,
                        2048 * g1 + 128 * j: 2048 * g1 + 128 * (j + 1)] = \
                        pre_w[r, f, :]
    for g1 in (2, 3):            # class B
        for g0 in range(4):
            for b in range(4):
                for rr in range(8):
                    f = 64 * g1 + 16 * g0 + 4 * b + rr // 2
                    wmm[32 * g0 + 8 * b + rr,
                        WB_B + 512 * (g1 - 2) + 128 * b:
                        WB_B + 512 * (g1 - 2) + 128 * (b + 1)] = \
                        pre_w[rr % 2, f, :]
    for g1 in range(4, 16):      # class C
        for g0 in range(4):
            for b in range(2):
                for rr in range(16):
                    f = 64 * g1 + 16 * g0 + 8 * b + rr // 2
                    wmm[32 * g0 + 16 * b + rr,
                        WB_C + 256 * (g1 - 4) + 128 * b:
                        WB_C + 256 * (g1 - 4) + 128 * (b + 1)] = \
                        pre_w[rr % 2, f, :]
    for r in range(2):           # runt
        wmm[r, WB_R:WB_R + 128] = pre_w[r, 1024, :]
    return wmm.astype(BF16)


def _build_bias_strip(pre_b):
    """(4, BSCOLS) bf16: per-off bank bias rows + trailing ones block."""
    bs = np.zeros((4, BSCOLS), dtype=np.float32)
    for o in range(4):
        banks = [q[o] for q in QUADS]
        if o == 0:
            banks = banks + [RUNT]
        for bk in banks:
            c = bk["bias_col"]
            for j, k in enumerate(bk["bands"]):
                bs[o, c + 128 * j: c + 128 * (j + 1)] = pre_b[k, :]
    bs[:, ONES_COL:ONES_COL + 128] = 1.0
    return bs.astype(BF16)


def _build_combo(xmm, wmm, bs):
    """(128, 1408) bf16: quad-0 working set in one transfer."""
    cb = np.zeros((128, 1408), dtype=BF16)
    cb[:, 0:256] = xmm[:, 0:256]
    cb[:, 256:768] = wmm[:, 0:512]
    for o in range(4):
        cb[32 * o, 768:1280] = bs[o, 0:512]
        cb[32 * o, 1280:1408] = np.array(1.0, dtype=BF16)
    return cb


def _build_xmm(x_core):
    """x_core (TOK, NF, IN_CH) -> (128, XCOLS) bf16 dense x.T layout."""
    xt = x_core.reshape(TOK, NF * IN_CH).T  # (2050, TOK)
    xp = np.zeros((NG * 128, TOK), dtype=np.float32)
    xp[:2050] = xt
    return np.ascontiguousarray(
        xp.reshape(NG, 128, TOK).transpose(1, 0, 2)).reshape(
            128, XCOLS).astype(BF16)


_INV_PERM = np.argsort(_BAND_PERM)


def _assemble(out_flat):
    """flat device output (bf16) -> (TOK, N_BANDS, OUT_CH) fp32.

    Device layout: chunk-major; h=0 uses the split ramp chunks
    (512, 3584, 7x4096, 128-runt), h=1 clean (8x4096, 128-runt).
    Stitch cols back together, then un-permute the band order."""
    half = out_flat.reshape(2, 128 * 32896)
    chunk_cols_h = ([512, 3584] + [4096] * 7 + [128],
                    [4096] * 8 + [128])
    oc = np.empty((TOK, N_BANDS, OUT_CH), dtype=np.float32)
    for h in range(2):
        flat = half[h]
        blk = np.empty((128, 32896), dtype=np.float32)
        c = 0
        o = 0
        for nc_ in chunk_cols_h[h]:
            blk[:, c:c + nc_] = flat[o:o + 128 * nc_].reshape(128, nc_)
            c += nc_
            o += 128 * nc_
        blk3 = blk.reshape(128, N_BANDS, OUT_CH)
        oc[128 * h:128 * (h + 1)] = blk3[:, _INV_PERM, :]
    return oc


# ---------------------------------------------------------------- device

_PROGRAM = None


def _build_program():
    global _PROGRAM
    if _PROGRAM is not None:
        return _PROGRAM

    nc = bacc.Bacc("TRN2", target_bir_lowering=False)
    f32 = mybir.dt.float32
    bf16 = mybir.dt.bfloat16
    xin = nc.dram_tensor("xmm", [128, XCOLS], bf16, kind="ExternalInput")
    win = nc.dram_tensor("wmm", [128, WCOLS], bf16, kind="ExternalInput")
    bsin = nc.dram_tensor("bs", [4, BSCOLS], bf16, kind="ExternalInput")
    cbin = nc.dram_tensor("combo", [128, 1408], bf16, kind="ExternalInput")
    out = nc.dram_tensor("out", [TOK * N_BANDS * OUT_CH], bf16,
                         kind="ExternalOutput")

    with TileContext(nc) as tc:
        with (
            tc.tile_pool(name="xw", bufs=1) as xw_pool,
            tc.tile_pool(name="stage", bufs=4) as stage_pool,
            tc.tile_pool(name="psum", bufs=4, space="PSUM") as psum_pool,
        ):
            x_sb = xw_pool.tile([128, XCOLS], bf16, tag="x")
            w_sb = xw_pool.tile([128, WCOLS], bf16, tag="w")
            bs_sb = xw_pool.tile([128, BSCOLS], bf16, tag="bs")
            cb_sb = xw_pool.tile([128, 1408], bf16, tag="cb")

            # Loads in first-needed-first order. The whole first chunk's
            # working set (x group 0, the A/g1=0/m=0 w block, quad-0 bias
            # rows + ones) arrives in ONE sync DMA (the "combo"), so the
            # first matmul waits on exactly one transfer. Everything else
            # streams on gpsimd (SWDGE) in quad order; the drain engines
            # and the output queue never issue loads.
            nc.sync.dma_start(out=cb_sb[:], in_=cbin.ap()[:])
            nc.sync.dma_start(out=x_sb[:, 0:256], in_=xin.ap()[:, 0:256])
            bs_dst = bs_sb[:].rearrange("(a b) c -> a b c", a=4)[:, 0:1, :]
            nc.gpsimd.dma_start(out=bs_dst,
                                in_=bsin.ap()[:].rearrange(
                                    "a (b c) -> a b c", b=1))
            nc.gpsimd.dma_start(out=w_sb[:, 512:2048],
                                in_=win.ap()[:, 512:2048])
            nc.gpsimd.dma_start(out=x_sb[:, 256:1024],
                                in_=xin.ap()[:, 256:1024])
            nc.gpsimd.dma_start(out=w_sb[:, 2048:5120],
                                in_=win.ap()[:, 2048:5120])
            nc.gpsimd.dma_start(out=x_sb[:, 1024:XCOLS],
                                in_=xin.ap()[:, 1024:XCOLS])
            nc.gpsimd.dma_start(out=w_sb[:, 5120:WCOLS],
                                in_=win.ap()[:, 5120:WCOLS])

            def emit_bank(ps, pc, bk, h, combo=False):
                off = bk["off"]
                tp = (off, 0)
                if combo:
                    ones = cb_sb[off:off + 1, 1280:1408]
                    bias = cb_sb[off:off + 1, 768:768 + bk["N"]]
                else:
                    ones = bs_sb[off:off + 1, ONES_COL:ONES_COL + 128]
                    bias = bs_sb[off:off + 1,
                                 bk["bias_col"]:bk["bias_col"] + bk["N"]]
                # prime bias (start resets PSUM)
                nc.tensor.matmul(
                    ps[:, pc:pc + bk["N"]], ones, bias,
                    start=True, stop=False, tile_position=tp,
                    skip_group_check=True,
                )
                c = pc
                for (r0, K, wcol, N) in bk["mms"]:
                    g = r0 // 128
                    row = r0 % 128
                    last = (c + N == pc + bk["N"])
                    if combo:
                        lhsT = cb_sb[row:row + K, 128 * h:128 * (h + 1)]
                        rhs = cb_sb[row:row + K, 256 + (wcol - 0):
                                    256 + (wcol - 0) + N]
                    else:
                        lhsT = x_sb[row:row + K, 256 * g + 128 * h:
                                    256 * g + 128 * (h + 1)]
                        rhs = w_sb[row:row + K, wcol:wcol + N]
                    nc.tensor.matmul(
                        ps[:, c:c + N], lhsT, rhs,
                        start=False, stop=last, tile_position=tp,
                        skip_group_check=True,
                    )
                    c += N

            # chunk grouping in PSUM-tile (2-bank, 1024-col) units:
            # first chunk = a single tile (first output DMA issues after
            # one drain), second = the next 3 tiles, then 4-tile (1 MB)
            # chunks. Tile t of quad q holds banks 2t, 2t+1 (offs
            # 0/32 | 64/96).
            tiles_flat = [(qi, ti) for qi in range(16) for ti in range(2)]
            o = 0
            drain_load = [0, 0]  # [dve_tiles, act_tiles]
            for h in range(HALVES):
                if h == 0:
                    # --- split first tile: chunk 0 = bank 0 alone, so
                    # the first output DMA issues after a single
                    # (128,512) drain. Only worth it on the ramp; h=1
                    # keeps clean 1 MB chunks (tiny chunks at the half
                    # boundary caused drain/PSUM-WAR stalls).
                    pt0 = psum_pool.tile([128, 1024], f32, tag="ps",
                                         name="pq")
                    emit_bank(pt0, 0, QUADS[0][0], h, combo=True)
                    sb0 = stage_pool.tile([128, 512], bf16, tag="st",
                                          name="s0")
                    nc.vector.tensor_copy(sb0[:], pt0[:, 0:512])
                    drain_load[0] += 1
                    nc.sync.dma_start(
                        out=out.ap()[o:o + 128 * 512]
                            .rearrange("(p n) -> p n", n=512),
                        in_=sb0[:],
                    )
                    o += 128 * 512
                    # --- chunk 1: second bank of tile 0 + tiles 1-3
                    emit_bank(pt0, 512, QUADS[0][1], h, combo=True)
                    sb1 = stage_pool.tile([128, 3584], bf16, tag="st",
                                          name="s1")
                    nc.scalar.copy(sb1[:, 0:512], pt0[:, 512:1024])
                    drain_load[1] += 1
                    for gi, (qi, ti) in enumerate(tiles_flat[1:4]):
                        quad = QUADS[qi]
                        pt = psum_pool.tile([128, 1024], f32, tag="ps",
                                            name="pq")
                        for bi in (2 * ti, 2 * ti + 1):
                            emit_bank(pt, 512 * (bi % 2), quad[bi], h,
                                      combo=(qi == 0))
                        if drain_load[0] * 1.21 <= drain_load[1] * 1.05:
                            drain_load[0] += 1
                            use_dve = True
                        else:
                            drain_load[1] += 1
                            use_dve = False
                        dst = sb1[:, 512 + 1024 * gi:512 + 1024 * (gi + 1)]
                        if use_dve:
                            nc.vector.tensor_copy(dst, pt[:])
                        else:
                            nc.scalar.copy(dst, pt[:])
                    nc.sync.dma_start(
                        out=out.ap()[o:o + 128 * 3584]
                            .rearrange("(p n) -> p n", n=3584),
                        in_=sb1[:],
                    )
                    o += 128 * 3584
                    groups = [tiles_flat[4 + 4 * i: 8 + 4 * i]
                              for i in range(7)]
                else:
                    groups = [tiles_flat[4 * i: 4 * i + 4]
                              for i in range(8)]
                # --- steady-state 1 MB chunks
                for grp in groups:
                    ncols = 1024 * len(grp)
                    sb = stage_pool.tile([128, ncols], bf16, tag="st",
                                         name="sq")
                    for gi, (qi, ti) in enumerate(grp):
                        quad = QUADS[qi]
                        pt = psum_pool.tile([128, 1024], f32, tag="ps",
                                            name="pq")
                        for bi in (2 * ti, 2 * ti + 1):
                            emit_bank(pt, 512 * (bi % 2), quad[bi], h,
                                      combo=(qi == 0))
                        # greedy balance by measured per-tile cost
                        # (DVE 1.21us, ACT 1.05us)
                        if drain_load[0] * 1.21 <= drain_load[1] * 1.05:
                            drain_load[0] += 1
                            use_dve = True
                        else:
                            drain_load[1] += 1
                            use_dve = False
                        dst = sb[:, 1024 * gi:1024 * (gi + 1)]
                        if use_dve:
                            nc.vector.tensor_copy(dst, pt[:])
                        else:
                            nc.scalar.copy(dst, pt[:])
                    nc.sync.dma_start(
                        out=out.ap()[o:o + 128 * ncols]
                            .rearrange("(p n) -> p n", n=ncols),
                        in_=sb[:],
                    )
                    o += 128 * ncols
                # runt bank
                ps = psum_pool.tile([128, 128], f32, tag="ps", name="pr")
                emit_bank(ps, 0, RUNT, h)
                sb = stage_pool.tile([128, 128], bf16, tag="st", name="sr")
                nc.scalar.copy(sb[:], ps[:])
                nc.sync.dma_start(
                    out=out.ap()[o:o + 128 * 128]
                        .rearrange("(p n) -> p n", n=128),
                    in_=sb[:],
                )
                o += 128 * 128

    nc.compile()
    _PROGRAM = nc
    return nc


# ---------------------------------------------------------------- entry

LAST_RESULTS = None


def kernel(x, pre_w, pre_b, _trace=False):
    global LAST_RESULTS
    x = np.asarray(x, dtype=np.float32)
    pre_w = np.asarray(pre_w, dtype=np.float32)
    pre_b = np.asarray(pre_b, dtype=np.float32)
    assert x.shape == (B, C, T, NF, IN_CH), x.shape

    nc = _build_program()
    wmm = _build_wmm(pre_w)
    bs = _build_bias_strip(pre_b)
    in_maps = []
    for core in range(N_CORES):
        b_, c_ = divmod(core, C)
        xmm = _build_xmm(x[b_, c_])
        in_maps.append({"xmm": xmm, "wmm": wmm, "bs": bs,
                        "combo": _build_combo(xmm, wmm, bs)})

    res = run_bass_kernel_spmd(
        nc, in_maps, core_ids=list(range(N_CORES)), trace=_trace,
    )
    LAST_RESULTS = res

    out = np.empty((B, C, T, N_BANDS, OUT_CH), dtype=np.float32)
    for core in range(N_CORES):
        b_, c_ = divmod(core, C)
        out[b_, c_] = _assemble(res.results[core]["out"])
    return out


# revision 9
# speedup vs baseline: 1.0320x; 1.0219x over previous
"""BandSplit kernel v4.5 for Trainium2 (8 NeuronCores, SPMD data-parallel).

Math: the (deterministic) melbank partitions the 1025 STFT bins into 257
contiguous segments (widths 1/4/8/8/1), all mel weights are 1.0, so

    out[b,c,t,k,o] = sum_{f in seg(k)} sum_i x[b,c,t,f,i]*pre_w[i,f,o] + pre_b[k,o]

Sharding: data-parallel over the 8 (b,c) pairs, one per core.
The stream is HBM-bound (358 GB/s/NC): ~20.1 MB/core (3.3 in + 16.8
out) vs 21.3 for the previous version; measured ~11% faster than it
under matched conditions (interleaved A/B on the same device state).

Design:
- x is loaded DENSE: xmm = x.T folded to (128, 17*256) bf16, 1.11 MB
  (previously packed per-mm copies + ones rows: 2.42 MB). Matmuls read
  full 32-row windows (K=32) of the dense layout; the block-diagonal
  band structure lives entirely in the w operand (32-aligned bases are
  a hard BIR-verifier rule, so w keeps its zero padding).
- bias is added by PSUM-priming matmuls (K=1 ones-row lhsT x bias-row
  rhs, start=True) before the data matmuls accumulate (start=False).
  Bias/ones rows live in a (4, ~9K) strip at partitions {0,32,64,96}.
- the ENTIRE first-chunk working set (x group 0, the A/g1=0 w block,
  quad-0 bias+ones rows) arrives in ONE "combo" DMA on sync, so the
  first matmul waits on exactly one transfer (~2 us DMA fixed latency
  dominates the ramp; serializing 3 loads cost ~2.5 us extra before).
- bulk loads stream on gpsimd (SWDGE) in quad order; Scalar/Vector do
  nothing but PSUM drains (greedy-balanced: ACT ~1.05 us vs DVE
  ~1.21 us per (128,1024) fp32->bf16 drain); sync does only output.
- output chunks: h=0 ramps with a single 512-col PSUM bank first (the
  first output DMA issues after one bank's matmuls + one small drain),
  then a 3.5-tile chunk, then 1 MB 4-tile chunks; h=1 is clean 8x1MB +
  runt (tiny chunks at the half boundary caused drain/PSUM-WAR stalls
  worth ~3 us; fixing them measured 63.7us vs 66.9 in matched A/B).
Rel err ~5.5e-3 (bf16 I/O rounding), inside the 2e-2 gate.
"""

import numpy as np
import ml_dtypes

import concourse.bacc as bacc
import concourse.mybir as mybir
from concourse.tile import TileContext
from concourse.bass_utils import run_bass_kernel_spmd

BF16 = np.dtype(ml_dtypes.bfloat16)

B, C, T, NF, IN_CH = 4, 2, 256, 1025, 2
N_BANDS, OUT_CH = 257, 128
N_CORES = 8
TOK = 256
HALVES = 2

NG = 17                    # dense x groups (2176 rows = 17*128)
XCOLS = NG * TOK           # 4352
WC_A, WC_B, WC_C = 4096, 1024, 3072
WB_B, WB_C, WB_R = WC_A, WC_A + WC_B, WC_A + WC_B + WC_C   # col bases
WCOLS = WC_A + WC_B + WC_C + 128                           # 8320

# ---------------------------------------------------------------- plan
# A bank: one mm  (window g1 in {0,1}, g0, m in 0..3): K=32 N=512, 4 bands
# B bank: one mm  (g1 in {2,3}, g0):                   K=32 N=512, 4 bands
# C bank: two mms (windows (g1,g0),(g1+1,g0)):         K=32 N=256 x2, 4 bands
# runt bank: one mm K=2 N=128 (band 256)
# bank = dict(off, mms=[(row0, K, wcol, N)...], bands=[...], N=512|128)


def _build_banks():
    by_off = [[] for _ in range(4)]
    for g1 in range(2):
        for m in range(4):
            for g0 in range(4):
                k0 = 64 * g1 + 16 * g0 + 4 * m
                by_off[g0].append(dict(
                    off=32 * g0,
                    mms=[(128 * g1 + 32 * g0, 32, 2048 * g1 + 512 * m, 512)],
                    bands=[k0, k0 + 1, k0 + 2, k0 + 3], N=512))
    for g1 in (2, 3):
        for g0 in range(4):
            k0 = 128 + 16 * (g1 - 2) + 4 * g0
            by_off[g0].append(dict(
                off=32 * g0,
                mms=[(128 * g1 + 32 * g0, 32, WB_B + 512 * (g1 - 2), 512)],
                bands=[k0, k0 + 1, k0 + 2, k0 + 3], N=512))
    for t in range(6):
        for g0 in range(4):
            g1a, g1b = 4 + 2 * t, 5 + 2 * t
            ka = 160 + 8 * (g1a - 4) + 2 * g0
            kb = 160 + 8 * (g1b - 4) + 2 * g0
            by_off[g0].append(dict(
                off=32 * g0,
                mms=[(128 * g1a + 32 * g0, 32, WB_C + 256 * (g1a - 4), 256),
                     (128 * g1b + 32 * g0, 32, WB_C + 256 * (g1b - 4), 256)],
                bands=[ka, ka + 1, kb, kb + 1], N=512))
    runt = dict(off=0, mms=[(128 * 16, 2, WB_R, 128)], bands=[256], N=128)
    # quads: rotate offs for PE row-group concurrency
    quads = []
    for i in range(16):
        quads.append([by_off[0][i], by_off[1][i], by_off[2][i], by_off[3][i]])
    by_off[0].append(runt)
    # bias strip cols per off
    bias_cols = [0, 0, 0, 0]
    for o in range(4):
        for bk in by_off[o]:
            bk["bias_col"] = bias_cols[o]
            bias_cols[o] += bk["N"]
    ones_col = max(bias_cols)
    bscols = ones_col + 128
    return quads, runt, bscols, ones_col


QUADS, RUNT, BSCOLS, ONES_COL = _build_banks()

# band order in output emission order (for host assembly)
_BAND_ORDER = []
for _h in range(HALVES):
    for _q in QUADS:
        for _bk in _q:
            _BAND_ORDER.extend(_bk["bands"])
    _BAND_ORDER.extend(RUNT["bands"])
_BAND_PERM = np.array(_BAND_ORDER[:N_BANDS * 1])  # same order per half
OELEMS = TOK * N_BANDS * OUT_CH

# ---------------------------------------------------------------- host prep


def _build_wmm(pre_w):
    """(128, WCOLS) bf16 block-diagonal weights (no bias rows)."""
    wmm = np.zeros((128, WCOLS), dtype=np.float32)
    for g1 in range(2):          # class A
        for g0 in range(4):
            for j in range(16):
                f = 64 * g1 + 16 * g0 + j
                for r in range(2):
                    wmm[32 * g0 + 2 * j + r,
                        2048 * g1 + 128 * j: 2048 * g1 + 128 * (j + 1)] = \
                        pre_w[r, f, :]
    for g1 in (2, 3):            # class B
        for g0 in range(4):
            for b in range(4):
                for rr in range(8):
                    f = 64 * g1 + 16 * g0 + 4 * b + rr // 2
                    wmm[32 * g0 + 8 * b + rr,
                        WB_B + 512 * (g1 - 2) + 128 * b:
                        WB_B + 512 * (g1 - 2) + 128 * (b + 1)] = \
                        pre_w[rr % 2, f, :]
    for g1 in range(4, 16):      # class C
        for g0 in range(4):
            for b in range(2):
                for rr in range(16):
                    f = 64 * g1 + 16 * g0 + 8 * b + rr // 2
                    wmm[32 * g0 + 16 * b + rr,
                        WB_C + 256 * (g1 - 4) + 128 * b:
                        WB_C + 256 * (g1 - 4) + 128 * (b + 1)] = \
                        pre_w[rr % 2, f, :]
    for r in range(2):           # runt
        wmm[r, WB_R:WB_R + 128] = pre_w[r, 1024, :]
    return wmm.astype(BF16)


def _build_bias_strip(pre_b):
    """(4, BSCOLS) bf16: per-off bank bias rows + trailing ones block."""
    bs = np.zeros((4, BSCOLS), dtype=np.float32)
    for o in range(4):
        banks = [q[o] for q in QUADS]
        if o == 0:
            banks = banks + [RUNT]
        for bk in banks:
            c = bk["bias_col"]
            for j, k in enumerate(bk["bands"]):
                bs[o, c + 128 * j: c + 128 * (j + 1)] = pre_b[k, :]
    bs[:, ONES_COL:ONES_COL + 128] = 1.0
    return bs.astype(BF16)


def _build_combo(xmm, wmm, bs):
    """(128, 1408) bf16: quad-0 working set in one transfer."""
    cb = np.zeros((128, 1408), dtype=BF16)
    cb[:, 0:256] = xmm[:, 0:256]
    cb[:, 256:768] = wmm[:, 0:512]
    for o in range(4):
        cb[32 * o, 768:1280] = bs[o, 0:512]
        cb[32 * o, 1280:1408] = np.array(1.0, dtype=BF16)
    return cb


def _build_xmm(x_core):
    """x_core (TOK, NF, IN_CH) -> (128, XCOLS) bf16 dense x.T layout."""
    xt = x_core.reshape(TOK, NF * IN_CH).T  # (2050, TOK)
    xp = np.zeros((NG * 128, TOK), dtype=np.float32)
    xp[:2050] = xt
    return np.ascontiguousarray(
        xp.reshape(NG, 128, TOK).transpose(1, 0, 2)).reshape(
            128, XCOLS).astype(BF16)


_INV_PERM = np.argsort(_BAND_PERM)


def _assemble(out_flat):
    """flat device output (bf16) -> (TOK, N_BANDS, OUT_CH) fp32.

    Device layout: chunk-major; h=0 uses the split ramp chunks
    (512, 3584, 7x4096, 128-runt), h=1 clean (8x4096, 128-runt).
    Stitch cols back together, then un-permute the band order."""
    half = out_flat.reshape(2, 128 * 32896)
    chunk_cols_h = ([512, 3584] + [4096] * 7 + [128],
                    [4096] * 8 + [128])
    oc = np.empty((TOK, N_BANDS, OUT_CH), dtype=np.float32)
    for h in range(2):
        flat = half[h]
        blk = np.empty((128, 32896), dtype=np.float32)
        c = 0
        o = 0
        for nc_ in chunk_cols_h[h]:
            blk[:, c:c + nc_] = flat[o:o + 128 * nc_].reshape(128, nc_)
            c += nc_
            o += 128 * nc_
        blk3 = blk.reshape(128, N_BANDS, OUT_CH)
        oc[128 * h:128 * (h + 1)] = blk3[:, _INV_PERM, :]
    return oc


# ---------------------------------------------------------------- device

_PROGRAM = None


def _build_program():
    global _PROGRAM
    if _PROGRAM is not None:
        return _PROGRAM

    nc = bacc.Bacc("TRN2", target_bir_lowering=False)
    f32 = mybir.dt.float32
    bf16 = mybir.dt.bfloat16
    xin = nc.dram_tensor("xmm", [128, XCOLS], bf16, kind="ExternalInput")
    win = nc.dram_tensor("wmm", [128, WCOLS], bf16, kind="ExternalInput")
    bsin = nc.dram_tensor("bs", [4, BSCOLS], bf16, kind="ExternalInput")
    cbin = nc.dram_tensor("combo", [128, 1408], bf16, kind="ExternalInput")
    mcin = nc.dram_tensor("mc", [32, 1408], bf16, kind="ExternalInput")
    out = nc.dram_tensor("out", [TOK * N_BANDS * OUT_CH], bf16,
                         kind="ExternalOutput")

    with TileContext(nc) as tc:
        with (
            tc.tile_pool(name="xw", bufs=1) as xw_pool,
            tc.tile_pool(name="stage", bufs=4) as stage_pool,
            tc.tile_pool(name="psum", bufs=4, space="PSUM") as psum_pool,
        ):
            x_sb = xw_pool.tile([128, XCOLS], bf16, tag="x")
            w_sb = xw_pool.tile([128, WCOLS], bf16, tag="w")
            bs_sb = xw_pool.tile([128, BSCOLS], bf16, tag="bs")
            cb_sb = xw_pool.tile([128, 1408], bf16, tag="cb")
            mc_sb = xw_pool.tile([32, 1408], bf16, tag="mc")

            # Loads in first-needed-first order. The whole first chunk's
            # working set (x group 0, the A/g1=0/m=0 w block, quad-0 bias
            # rows + ones) arrives in ONE sync DMA (the "combo"), so the
            # first matmul waits on exactly one transfer. Everything else
            # streams on gpsimd (SWDGE) in quad order; the drain engines
            # and the output queue never issue loads.
            # mini-combo (32 partitions, ~90KB) = just the FIRST bank's
            # operands; its DMA completes ~0.8us before the full combo,
            # pulling the whole ramp chain earlier.
            nc.sync.dma_start(out=mc_sb[:], in_=mcin.ap()[:])
            nc.sync.dma_start(out=cb_sb[:], in_=cbin.ap()[:])
            nc.sync.dma_start(out=x_sb[:, 0:256], in_=xin.ap()[:, 0:256])
            bs_dst = bs_sb[:].rearrange("(a b) c -> a b c", a=4)[:, 0:1, :]
            nc.gpsimd.dma_start(out=bs_dst,
                                in_=bsin.ap()[:].rearrange(
                                    "a (b c) -> a b c", b=1))
            nc.gpsimd.dma_start(out=w_sb[:, 512:2048],
                                in_=win.ap()[:, 512:2048])
            nc.gpsimd.dma_start(out=x_sb[:, 256:1024],
                                in_=xin.ap()[:, 256:1024])
            nc.gpsimd.dma_start(out=w_sb[:, 2048:5120],
                                in_=win.ap()[:, 2048:5120])
            nc.gpsimd.dma_start(out=x_sb[:, 1024:XCOLS],
                                in_=xin.ap()[:, 1024:XCOLS])
            nc.gpsimd.dma_start(out=w_sb[:, 5120:WCOLS],
                                in_=win.ap()[:, 5120:WCOLS])

            def emit_bank(ps, pc, bk, h, combo=False):
                off = bk["off"]
                tp = (off, 0)
                if combo:
                    ones = cb_sb[off:off + 1, 1280:1408]
                    bias = cb_sb[off:off + 1, 768:768 + bk["N"]]
                else:
                    ones = bs_sb[off:off + 1, ONES_COL:ONES_COL + 128]
                    bias = bs_sb[off:off + 1,
                                 bk["bias_col"]:bk["bias_col"] + bk["N"]]
                # prime bias (start resets PSUM)
                nc.tensor.matmul(
                    ps[:, pc:pc + bk["N"]], ones, bias,
                    start=True, stop=False, tile_position=tp,
                    skip_group_check=True,
                )
                c = pc
                for (r0, K, wcol, N) in bk["mms"]:
                    g = r0 // 128
                    row = r0 % 128
                    last = (c + N == pc + bk["N"])
                    if combo:
                        lhsT = cb_sb[row:row + K, 128 * h:128 * (h + 1)]
                        rhs = cb_sb[row:row + K, 256 + (wcol - 0):
                                    256 + (wcol - 0) + N]
                    else:
                        lhsT = x_sb[row:row + K, 256 * g + 128 * h:
                                    256 * g + 128 * (h + 1)]
                        rhs = w_sb[row:row + K, wcol:wcol + N]
                    nc.tensor.matmul(
                        ps[:, c:c + N], lhsT, rhs,
                        start=False, stop=last, tile_position=tp,
                        skip_group_check=True,
                    )
                    c += N

            # chunk grouping in PSUM-tile (2-bank, 1024-col) units:
            # first chunk = a single tile (first output DMA issues after
            # one drain), second = the next 3 tiles, then 4-tile (1 MB)
            # chunks. Tile t of quad q holds banks 2t, 2t+1 (offs
            # 0/32 | 64/96).
            tiles_flat = [(qi, ti) for qi in range(16) for ti in range(2)]
            o = 0
            drain_load = [0, 0]  # [dve_tiles, act_tiles]
            for h in range(HALVES):
                if h == 0:
                    # --- split first tile: chunk 0 = bank 0 alone, so
                    # the first output DMA issues after a single
                    # (128,512) drain. Only worth it on the ramp; h=1
                    # keeps clean 1 MB chunks (tiny chunks at the half
                    # boundary caused drain/PSUM-WAR stalls).
                    pt0 = psum_pool.tile([128, 1024], f32, tag="ps",
                                         name="pq")
                    bk0 = QUADS[0][0]
                    # first bank reads the mini-combo (rows 0..31 only)
                    nc.tensor.matmul(
                        pt0[:, 0:512],
                        mc_sb[0:1, 1280:1408],
                        mc_sb[0:1, 768:768 + 512],
                        start=True, stop=False, tile_position=(0, 0),
                        skip_group_check=True,
                    )
                    (r0, K, wcol, N) = bk0["mms"][0]
                    nc.tensor.matmul(
                        pt0[:, 0:512],
                        mc_sb[0:K, 128 * h:128 * (h + 1)],
                        mc_sb[0:K, 256 + wcol:256 + wcol + N],
                        start=False, stop=True, tile_position=(0, 0),
                        skip_group_check=True,
                    )
                    sb0 = stage_pool.tile([128, 512], bf16, tag="st",
                                          name="s0")
                    nc.vector.tensor_copy(sb0[:], pt0[:, 0:512])
                    drain_load[0] += 1
                    nc.sync.dma_start(
                        out=out.ap()[o:o + 128 * 512]
                            .rearrange("(p n) -> p n", n=512),
                        in_=sb0[:],
                    )
                    o += 128 * 512
                    # --- chunk 1: second bank of tile 0 + tiles 1-3
                    emit_bank(pt0, 512, QUADS[0][1], h, combo=True)
                    sb1 = stage_pool.tile([128, 3584], bf16, tag="st",
                                          name="s1")
                    nc.scalar.copy(sb1[:, 0:512], pt0[:, 512:1024])
                    drain_load[1] += 1
                    for gi, (qi, ti) in enumerate(tiles_flat[1:4]):
                        quad = QUADS[qi]
                        pt = psum_pool.tile([128, 1024], f32, tag="ps",
                                            name="pq")
                        for bi in (2 * ti, 2 * ti + 1):
                            emit_bank(pt, 512 * (bi % 2), quad[bi], h,
                                      combo=(qi == 0))
                        if drain_load[0] * 1.21 <= drain_load[1] * 1.05:
                            drain_load[0] += 1
                            use_dve = True
                        else:
                            drain_load[1] += 1
                            use_dve = False
                        dst = sb1[:, 512 + 1024 * gi:512 + 1024 * (gi + 1)]
                        if use_dve:
                            nc.vector.tensor_copy(dst, pt[:])
                        else:
                            nc.scalar.copy(dst, pt[:])
                    nc.sync.dma_start(
                        out=out.ap()[o:o + 128 * 3584]
                            .rearrange("(p n) -> p n", n=3584),
                        in_=sb1[:],
                    )
                    o += 128 * 3584
                    groups = [tiles_flat[4 + 4 * i: 8 + 4 * i]
                              for i in range(7)]
                else:
                    groups = [tiles_flat[4 * i: 4 * i + 4]
                              for i in range(8)]
                # --- steady-state 1 MB chunks
                for grp in groups:
                    ncols = 1024 * len(grp)
                    sb = stage_pool.tile([128, ncols], bf16, tag="st",
                                         name="sq")
                    for gi, (qi, ti) in enumerate(grp):
                        quad = QUADS[qi]
                        pt = psum_pool.tile([128, 1024], f32, tag="ps",
                                            name="pq")
                        for bi in (2 * ti, 2 * ti + 1):
                            emit_bank(pt, 512 * (bi % 2), quad[bi], h,
                                      combo=(qi == 0))
                        # greedy balance by measured per-tile cost
                        # (DVE 1.21us, ACT 1.05us)
                        if drain_load[0] * 1.21 <= drain_load[1] * 1.05:
                            drain_load[0] += 1
                            use_dve = True
                        else:
                            drain_load[1] += 1
                            use_dve = False
                        dst = sb[:, 1024 * gi:1024 * (gi + 1)]
                        if use_dve:
                            nc.vector.tensor_copy(dst, pt[:])
                        else:
                            nc.scalar.copy(dst, pt[:])
                    nc.sync.dma_start(
                        out=out.ap()[o:o + 128 * ncols]
                            .rearrange("(p n) -> p n", n=ncols),
                        in_=sb[:],
                    )
                    o += 128 * ncols
                # runt bank
                ps = psum_pool.tile([128, 128], f32, tag="ps", name="pr")
                emit_bank(ps, 0, RUNT, h)
                sb = stage_pool.tile([128, 128], bf16, tag="st", name="sr")
                nc.scalar.copy(sb[:], ps[:])
                nc.sync.dma_start(
                    out=out.ap()[o:o + 128 * 128]
                        .rearrange("(p n) -> p n", n=128),
                    in_=sb[:],
                )
                o += 128 * 128

    nc.compile()
    _PROGRAM = nc
    return nc


# ---------------------------------------------------------------- entry

LAST_RESULTS = None


def kernel(x, pre_w, pre_b, _trace=False):
    global LAST_RESULTS
    x = np.asarray(x, dtype=np.float32)
    pre_w = np.asarray(pre_w, dtype=np.float32)
    pre_b = np.asarray(pre_b, dtype=np.float32)
    assert x.shape == (B, C, T, NF, IN_CH), x.shape

    nc = _build_program()
    wmm = _build_wmm(pre_w)
    bs = _build_bias_strip(pre_b)
    in_maps = []
    for core in range(N_CORES):
        b_, c_ = divmod(core, C)
        xmm = _build_xmm(x[b_, c_])
        cb = _build_combo(xmm, wmm, bs)
        in_maps.append({"xmm": xmm, "wmm": wmm, "bs": bs,
                        "combo": cb, "mc": np.ascontiguousarray(cb[0:32])})

    res = run_bass_kernel_spmd(
        nc, in_maps, core_ids=list(range(N_CORES)), trace=_trace,
    )
    LAST_RESULTS = res

    out = np.empty((B, C, T, N_BANDS, OUT_CH), dtype=np.float32)
    for core in range(N_CORES):
        b_, c_ = divmod(core, C)
        out[b_, c_] = _assemble(res.results[core]["out"])
    return out
